# revision 1
# baseline (speedup 1.0000x reference)
"""Multi-head attention (B=8, S=2048, D=512, H=8) on 8 Trainium2 NeuronCores.

Strategy: pure data parallelism — one batch element per core, no collectives.

Per-core device pipeline (all matmuls fp16 with fp32 PSUM accumulation):
  1. Projections: qT/kT in transposed layout [e, s] (attention contracts
     dk on partitions), v in natural [s, e] layout augmented with a ones
     column per head (the PV matmul then also produces softmax denominators).
     Inputs arrive pre-transposed from host as X^T [c, s] fp16.
  2. Attention per (head, s-half): scoresT[j, s] = (kT_h slice).T @ qT_h,
     exp on ScalarE (PSUM->SBUF fp16), multiplicative 0/1 mask (DVE fp16 2x),
     PV matmul with [V|1] stationary accumulating outT rows + denom in PSUM.
  3. DVE reciprocal of denominators, partition-broadcast via a DRAM bounce,
     normalize, final projection with Wo.T, bias, DMA out.

The final projection is folded into the attention phase: each s-half is
normalized and projected as soon as its denominators are complete, so the
tail after the last attention block is short.

Softmax note: reference softmax(where(mask==0, -1e30, s)) == exp(s)*mask
normalized — scores are O(1) so no max-subtraction is needed, and the 0/1
mask is exact in fp16. Scale 1/sqrt(dk)=1/8 is folded into Wq/bq on host.
"""
import numpy as np

import concourse.bacc as bacc
import concourse.bass as bass
import concourse.mybir as mybir
import concourse.tile as tile
from concourse.bass_utils import run_bass_kernel_spmd

B, S, D, H, DK = 8, 2048, 512, 8, 64
P = 128            # partition tile
NET = D // P       # 4 e-tiles (contraction chunks / head pairs)
NST = S // P       # 16 s-tiles / j-tiles
SCW = 512          # matmul moving free dim
NSC = S // SCW     # 4
SHW = 1024         # attention s-block width (2 PSUM banks)
NSH = S // SHW     # 2

f32 = mybir.dt.float32
fp16 = mybir.dt.float16

_CACHE: dict = {}


def _build():
    nc = bacc.Bacc("TRN2", target_bir_lowering=False, debug=False)

    d_xq = nc.dram_tensor("xq", [D, S], fp16, kind="ExternalInput")
    d_xk = nc.dram_tensor("xk", [D, S], fp16, kind="ExternalInput")
    d_xv = nc.dram_tensor("xv", [D, S], fp16, kind="ExternalInput")
    d_mskT = nc.dram_tensor("mskT", [S, S], fp16, kind="ExternalInput")
    d_wq = nc.dram_tensor("wq", [D, D], fp16, kind="ExternalInput")  # Wq.T/8
    d_wk = nc.dram_tensor("wk", [D, D], fp16, kind="ExternalInput")  # Wk.T
    d_wv = nc.dram_tensor("wv", [D, D], fp16, kind="ExternalInput")  # Wv.T
    d_wo = nc.dram_tensor("wo", [D, D], fp16, kind="ExternalInput")  # Wo.T
    d_bq = nc.dram_tensor("bq", [D], f32, kind="ExternalInput")      # bq/8
    d_bk = nc.dram_tensor("bk", [D], f32, kind="ExternalInput")
    d_bv = nc.dram_tensor("bv", [D], f32, kind="ExternalInput")
    d_bo = nc.dram_tensor("bo", [D], f32, kind="ExternalInput")
    d_out = nc.dram_tensor("out", [S, D], f32, kind="ExternalOutput")
    d_rec = nc.dram_tensor("rec_dram", [H, S], f32)

    Exp = mybir.ActivationFunctionType.Exp

    with tile.TileContext(nc) as tc, \
         tc.tile_pool(name="persist", bufs=1) as persist:

        qT = persist.tile([P, NET, S], fp16)             # [e%128, et, s]
        kT = persist.tile([P, NET, S], fp16)
        v_aug = persist.tile([P, NST, H, DK + 1], fp16)  # [j%128, jt, h, d|1]
        outT = persist.tile([P, NET, S], fp16)           # [hd%128, et, s] unnorm
        denom = persist.tile([P, NSH, 64], f32)
        bq_sb = persist.tile([P, NET], f32)
        bk_sb = persist.tile([P, NET], f32)
        bv_bc = persist.tile([P, D], f32)
        wo_sb = persist.tile([P, NET, D], fp16)
        bo_bc = persist.tile([P, D], f32)
        outTn = persist.tile([P, NET, S], fp16)

        nc.sync.dma_start(out=bq_sb, in_=d_bq.ap().rearrange("(cc p) -> p cc", p=P))
        nc.sync.dma_start(out=bk_sb, in_=d_bk.ap().rearrange("(cc p) -> p cc", p=P))
        nc.sync.dma_start(
            out=bv_bc,
            in_=bass.AP(tensor=d_bv.ap().tensor, offset=0, ap=[[0, P], [1, D]]))
        nc.vector.memset(v_aug[:, :, :, DK:DK + 1], 1.0)

        with tc.tile_pool(name="maskp", bufs=1) as maskp:
          maskT = maskp.tile([P, NST, S], fp16)
          msk_ap = d_mskT.ap().rearrange("(jt p) s -> p jt s", p=P)

          # ---------------- projections (q, k, v) ----------------
          with tc.tile_pool(name="projx", bufs=2) as projx, \
               tc.tile_pool(name="projw", bufs=2) as projw, \
               tc.tile_pool(name="projps", bufs=4, space="PSUM") as projps:
            mask_sched = {0: range(0, 4), 1: range(4, 8), 2: range(8, NST)}
            for which, (d_x, d_w) in enumerate(
                    [(d_xq, d_wq), (d_xk, d_wk), (d_xv, d_wv)]):
                w_sb = projw.tile([P, NET, D], fp16, tag="w", name="w_sb")
                nc.sync.dma_start(
                    out=w_sb, in_=d_w.ap().rearrange("(cc p) e -> p cc e", p=P))
                x_sb = projx.tile([P, NET, S], fp16, tag="x", name="x_sb")
                x_ap = d_x.ap().rearrange("(cc p) s -> p cc s", p=P)
                for cc in range(NET):
                    nc.sync.dma_start(out=x_sb[:, cc, :], in_=x_ap[:, cc, :])
                if which == 0:
                    nc.sync.dma_start(
                        out=wo_sb,
                        in_=d_wo.ap().rearrange("(cc p) e -> p cc e", p=P))
                    nc.sync.dma_start(
                        out=bo_bc,
                        in_=bass.AP(tensor=d_bo.ap().tensor, offset=0,
                                    ap=[[0, P], [1, D]]))
                for jt in mask_sched[which]:
                    nc.sync.dma_start(out=maskT[:, jt, :], in_=msk_ap[:, jt, :])

                if which == 2:  # v -> natural layout [s, e] into v_aug
                    for st in range(NST):
                        ps_t = projps.tile([P, SCW], f32, tag="ps",
                                           name="ps_t")
                        for cc in range(NET):
                            nc.tensor.matmul(
                                ps_t,
                                x_sb[:, cc, st * P:(st + 1) * P],
                                w_sb[:, cc, :],
                                start=(cc == 0), stop=(cc == NET - 1))
                        nc.vector.tensor_add(
                            v_aug[:, st, :, 0:DK],
                            ps_t.rearrange("p (h d) -> p h d", h=H),
                            bv_bc.rearrange("p (h d) -> p h d", h=H))
                else:  # q, k -> transposed layout [e, s]
                    dst = qT if which == 0 else kT
                    bias = bq_sb if which == 0 else bk_sb
                    for et in range(NET):
                        for sc in range(NSC):
                            ps_t = projps.tile([P, SCW], f32, tag="ps",
                                               name="ps_t")
                            for cc in range(NET):
                                nc.tensor.matmul(
                                    ps_t,
                                    w_sb[:, cc, et * P:(et + 1) * P],
                                    x_sb[:, cc, sc * SCW:(sc + 1) * SCW],
                                    start=(cc == 0), stop=(cc == NET - 1))
                            nc.scalar.activation(
                                dst[:, et, sc * SCW:(sc + 1) * SCW], ps_t,
                                mybir.ActivationFunctionType.Identity,
                                bias=bias[:, et:et + 1])

          # ---------------- attention ----------------
          # sh outer: when the first s-half of all heads is done, its
          # reciprocal + normalize run while the second half computes.
          with tc.tile_pool(name="attn", bufs=4) as attn, \
               tc.tile_pool(name="attnps", bufs=2, space="PSUM") as attnps:
            for sh in range(NSH):
                c0 = sh * SHW
                for h in range(H):
                    et, ro = h // 2, 64 * (h % 2)
                    pv0 = attnps.tile([65, SCW], f32, tag="pv", bufs=3,
                                      name="pv0")
                    pv1 = attnps.tile([65, SCW], f32, tag="pv", bufs=3,
                                      name="pv1")
                    pvs = (pv0, pv1)
                    for jt in range(NST):
                        sc_ps = attnps.tile([P, SHW], f32, tag="sc",
                                            bufs=2, name="sc_ps")
                        for i in range(2):
                            nc.tensor.matmul(
                                sc_ps[:, i * SCW:(i + 1) * SCW],
                                kT[ro:ro + DK, et, jt * P:(jt + 1) * P],
                                qT[ro:ro + DK, et,
                                   c0 + i * SCW:c0 + (i + 1) * SCW],
                                start=True, stop=True)
                        ex = attn.tile([P, SHW], fp16, tag="ex", bufs=6, name="ex")
                        nc.scalar.activation(ex, sc_ps, Exp)
                        pb = attn.tile([P, SHW], fp16, tag="pb", bufs=6, name="pb")
                        nc.vector.tensor_mul(
                            pb, ex, maskT[:, jt, c0:c0 + SHW])
                        for i in range(2):
                            nc.tensor.matmul(
                                pvs[i], v_aug[:, jt, h, :],
                                pb[:, i * SCW:(i + 1) * SCW],
                                start=(jt == 0), stop=(jt == NST - 1))
                    for i in range(2):
                        cols = c0 + i * SCW
                        nc.vector.tensor_copy(
                            outT[ro:ro + DK, et, cols:cols + SCW],
                            pvs[i][0:DK, :])
                        dst_t = attn.tile([65, SCW], f32, tag="dst", bufs=2,
                                          name="dst_t")
                        nc.vector.tensor_copy(dst_t[64:65, :], pvs[i][64:65, :])
                        pbase = h * 16 + i * 8
                        nc.gpsimd.dma_start(
                            out=denom[pbase:pbase + 8, sh, :],
                            in_=dst_t[64:65, :])
                    if h % 2 == 1:
                        # pair (2et, 2et+1) done for this half: normalize now
                        rec = attn.tile([32, 64], f32, tag="rec", bufs=2,
                                        name="rec")
                        nc.vector.reciprocal(
                            rec, denom[et * 32:(et + 1) * 32, sh, :])
                        nc.sync.dma_start(
                            out=d_rec.ap()[2 * et:2 * et + 2, c0:c0 + SHW],
                            in_=rec)
                        rb = attn.tile([P, SHW], f32, tag="rb", bufs=2,
                                       name="rb")
                        nc.gpsimd.dma_start(
                            out=rb[0:64, :],
                            in_=bass.AP(tensor=d_rec.ap().tensor,
                                        offset=(2 * et) * S + c0,
                                        ap=[[0, 64], [1, SHW]]))
                        nc.gpsimd.dma_start(
                            out=rb[64:128, :],
                            in_=bass.AP(tensor=d_rec.ap().tensor,
                                        offset=(2 * et + 1) * S + c0,
                                        ap=[[0, 64], [1, SHW]]))
                        nc.vector.tensor_mul(outTn[:, et, c0:c0 + SHW],
                                             outT[:, et, c0:c0 + SHW], rb)
                # (normalization now happens per head pair, inline above)
                # final projection for this s-half (PSUM slots shared with pv)
                for st in range(sh * NST // NSH, (sh + 1) * NST // NSH):
                    ps_f = attnps.tile([P, D], f32, tag="pf", bufs=1,
                                       name="ps_f")
                    for cc in range(NET):
                        nc.tensor.matmul(
                            ps_f,
                            outTn[:, cc, st * P:(st + 1) * P],
                            wo_sb[:, cc, :],
                            start=(cc == 0), stop=(cc == NET - 1))
                    o_sb = attn.tile([P, D], f32, tag="os", bufs=2,
                                     name="o_sb")
                    nc.vector.tensor_add(o_sb, ps_f, bo_bc)
                    nc.sync.dma_start(
                        out=d_out.ap()[st * P:(st + 1) * P, :], in_=o_sb)

    nc.compile()
    return nc


def _get_nc():
    if "nc" not in _CACHE:
        _CACHE["nc"] = _build()
    return _CACHE["nc"]


def _preprocess(Q, K, V, mask, Wq, bq, Wk, bk, Wv, bv, Wo, bo):
    """Host-side sharding + layout marshalling (per-core input dicts)."""
    mT = np.ascontiguousarray(np.asarray(mask)[0, 0].T).astype(np.float16)
    wq_h = np.ascontiguousarray(np.asarray(Wq).T / 8.0).astype(np.float16)
    wk_h = np.ascontiguousarray(np.asarray(Wk).T).astype(np.float16)
    wv_h = np.ascontiguousarray(np.asarray(Wv).T).astype(np.float16)
    wo_h = np.ascontiguousarray(np.asarray(Wo).T).astype(np.float16)
    bq_h = np.asarray(bq, dtype=np.float32) / 8.0
    bk_h = np.asarray(bk, dtype=np.float32)
    bv_h = np.asarray(bv, dtype=np.float32)
    bo_h = np.asarray(bo, dtype=np.float32)
    Q, K, V = np.asarray(Q), np.asarray(K), np.asarray(V)
    in_maps = []
    for b in range(B):
        in_maps.append({
            "xq": np.ascontiguousarray(Q[b].T).astype(np.float16),
            "xk": np.ascontiguousarray(K[b].T).astype(np.float16),
            "xv": np.ascontiguousarray(V[b].T).astype(np.float16),
            "mskT": mT,
            "wq": wq_h, "wk": wk_h, "wv": wv_h, "wo": wo_h,
            "bq": bq_h, "bk": bk_h, "bv": bv_h, "bo": bo_h,
        })
    return in_maps


def run(inputs: dict, trace: bool = False):
    nc = _get_nc()
    in_maps = _preprocess(**inputs)
    res = run_bass_kernel_spmd(nc, in_maps, core_ids=list(range(B)), trace=trace)
    outp = np.stack([res.results[b]["out"] for b in range(B)], axis=0)
    return outp.astype(np.float32), res


def kernel(**inputs) -> np.ndarray:
    outp, _ = run(inputs, trace=False)
    return outp

